# revision 35
# baseline (speedup 1.0000x reference)
"""Caser forward pass on 8 Trainium2 NeuronCores.

Sharding: data-parallel over batch (128 rows/core) for the embedding gather,
convs and FC layers; the tied-weight logits matmul is vocab-sharded (each
core keeps a 12500-item fp16 slice of the embedding table resident in SBUF,
receives every core's seq_out via AllGather, and writes logits[:, its slice]).
The per-core batch is processed in NSLICE slices, and slice s's logits are
emitted after slice s+1's conv so the AllGather latency and the logits
out-DMA hide under conv matmuls.

Horizontal convs are computed as shift-accumulated matmuls: for each group g
of 8 kernel heights (8*16=128 output rows = (k, n) pairs) and each shift j,
one bf16 matmul lhsT=W[g,j] [d=128, 128 rows], rhs=XT[:, (b, t+j)] slice,
PSUM-accumulated over j. A final rank-8 "mask" matmul adds -30000 to the
(row k, t >= 50-k) staircase so a single full-width reduce_max per chunk
implements the per-k valid-range max-pool. All conv/fc weights are bf16 and
stay resident in SBUF; small-T groups (HOIST_G) are computed up front for
the whole 128-row batch to keep matmul free dims large.

The vertical conv is folded into fc1 on the host: G[l][d, D'] =
sum_v fc1_w[D', v*128+d] * conv_v_w[v, l]; its contribution zv is computed
once for all 128 local rows, and per-slice fc1 adds the pooled part.
"""

import os

import numpy as np

B, L, D = 1024, 50, 128
NH, NV = 16, 4
N_ITEMS, N_USERS = 100000, 50000
NCORES = 8
BL = B // NCORES  # 128 batch rows per core
NG = 7  # ceil(50/8) groups of kernel heights
MASK_VAL = -30000.0  # must be representable in bf16/fp16
NSLICE = 4  # batch slices per core: logits(s) overlaps conv(s+1)
SB = BL // NSLICE  # batch rows per slice
IPC = N_ITEMS // NCORES  # 12500 vocab items per core (vocab-sharded logits)
SUB = 512  # logits matmul free dim (one PSUM bank)
OCHUNK = 2048  # logits out-DMA chunk width
CONV_PSUM = 512  # this walrus caps the matmul moving free dim at 512
HOIST_G = (4, 5, 6)  # small-T groups computed up front for the whole batch
SWDGE_QUEUES = 4  # gpsimd SWDGE queues (gathers serialize per queue)

# filled for test.py's "HW exec time" reporting
LAST_EXEC_NS = None
# when True, _build_program adds intermediate-dump outputs (debugging only)
DEBUG_DUMPS = False
# sim-experiment switches (leave False in production)
SKIP_LOGITS = False
# shared with the walrus wait-splitting patch (installed once, reused)
_CURRENT_NC = [None]


def _group_geometry():
    geo = []
    for g in range(NG):
        T = 50 - 8 * g
        jmax = min(8 * (g + 1), 50)
        geo.append((T, jmax))
    return geo


GEO = _group_geometry()
WSTACK_STARTS = []
_n = 0
for _g in range(NG):
    WSTACK_STARTS.append(_n)
    _n += GEO[_g][1]
WSTACK_N = _n  # 218


def _prep_host(inputs):
    """Build all derived weight layouts on the host (numpy only)."""
    f32, f16 = np.float32, np.float16
    bf16 = f16  # fp16 everywhere: walrus caps moving N at 512 for all dtypes
    seq = np.asarray(inputs["seq"]).astype(np.int32).reshape(B, L)
    user_id = np.asarray(inputs["user_id"]).astype(np.int32).reshape(B, 1)
    item_emb = np.ascontiguousarray(np.asarray(inputs["item_emb"], dtype=f32))
    user_emb = np.ascontiguousarray(np.asarray(inputs["user_emb_table"], dtype=f32))
    conv_v_w = np.asarray(inputs["conv_v_w"], dtype=f32)  # [NV, L]
    conv_v_b = np.asarray(inputs["conv_v_b"], dtype=f32)  # [NV]
    conv_h_ws = [np.asarray(w, dtype=f32) for w in inputs["conv_h_ws"]]
    conv_h_bs = np.asarray(inputs["conv_h_bs"], dtype=f32)  # [L, NH]
    fc1_w = np.asarray(inputs["fc1_w"], dtype=f32)
    fc1_b = np.asarray(inputs["fc1_b"], dtype=f32)
    fc2_w = np.asarray(inputs["fc2_w"], dtype=f32)
    fc2_b = np.asarray(inputs["fc2_b"], dtype=f32)

    # conv weight stack: wstack[idx(g,j)][d, kk*16+n] = conv_h_ws[8g+kk][n, j, d]
    wstack = np.zeros((WSTACK_N, D, 128), bf16)
    for g in range(NG):
        T, jmax = GEO[g]
        for j in range(jmax):
            idx = WSTACK_STARTS[g] + j
            for kk in range(8):
                k = 8 * g + kk
                if k >= L or j > k:
                    continue
                wstack[idx][:, kk * 16 : (kk + 1) * 16] = conv_h_ws[k][
                    :, j, :
                ].T.astype(bf16)
    # mask: lhsT[g][d', m] = MASK_VAL where m//16 == d'; rhs[g][d', b*T+t] = 1
    # where t >= 50-(8g+d')  ->  product = MASK_VAL at (row k, t >= 50-k)
    mlhs = np.zeros((NG, D, 128), bf16)
    mrhs = np.zeros((NG, D, CONV_PSUM), bf16)
    for g in range(NG):
        T, _ = GEO[g]
        cbmax = min(BL, CONV_PSUM // T)
        for kk in range(8):
            k = 8 * g + kk
            if k >= L:
                continue
            mlhs[g][kk, kk * 16 : (kk + 1) * 16] = MASK_VAL
            tlo = L - k
            if tlo < T:
                pat = np.zeros(T, f32)
                pat[tlo:] = 1.0
                mrhs[g][kk, : cbmax * T] = np.tile(pat, cbmax)[: cbmax * T].astype(
                    bf16
                )
    # vertical conv folded into fc1
    fc1_wv = fc1_w[:, : NV * D].reshape(D, NV, D)  # [o, v, d]
    gmat = np.ascontiguousarray(
        np.einsum("ovd,vl->ldo", fc1_wv, conv_v_w).astype(bf16)
    )  # [L, d, o]
    bias1 = fc1_b + np.einsum("ovd,v->o", fc1_wv, conv_v_b)  # [D]
    f1h = np.zeros((NG, 128, D), bf16)
    fc1_wh = fc1_w[:, NV * D :]  # [D, 800]
    for g in range(NG):
        r0 = g * 128
        r1 = min(r0 + 128, NH * L)
        f1h[g][: r1 - r0, :] = fc1_wh[:, r0:r1].T.astype(bf16)
    hbias = np.zeros((128, NG), f32)
    for g in range(NG):
        for kk in range(8):
            k = 8 * g + kk
            if k >= L:
                continue
            hbias[kk * 16 : (kk + 1) * 16, g] = conv_h_bs[k]
    f2z = np.ascontiguousarray(fc2_w[:, :D].T.astype(bf16))  # [z, o]
    f2u = np.ascontiguousarray(fc2_w[:, D:].T.astype(bf16))  # [u, o]
    biases = np.zeros((128, 9), f32)
    biases[:, :NG] = hbias
    biases[:, 7] = bias1
    biases[:, 8] = fc2_b
    embt16 = np.ascontiguousarray(item_emb.T.astype(f16))  # [D, N_ITEMS]

    shared = {
        "item_emb": item_emb,
        "user_emb": user_emb,
        "wstack": wstack,
        "mlhs": mlhs,
        "mrhs": mrhs,
        "gmat": gmat,
        "f1h": f1h,
        "f2z": f2z,
        "f2u": f2u,
        "biases": biases,
    }
    in_maps = []
    for c in range(NCORES):
        m = dict(shared)
        m["seq_i"] = np.ascontiguousarray(seq[c * BL : (c + 1) * BL])
        m["uid_i"] = np.ascontiguousarray(user_id[c * BL : (c + 1) * BL])
        m["embt16"] = np.ascontiguousarray(embt16[:, c * IPC : (c + 1) * IPC])
        in_maps.append(m)
    return in_maps


def _install_walrus_workarounds(mybir, TileContext, TileClockWait, ScopedClock):
    """This walrus build rejects >1 sync wait per instruction. Split
    multi-wait instructions into single-wait NOPs on the same engine, and
    rebuild the TileContext exit drain the same way."""
    current_nc = _CURRENT_NC
    if getattr(TileClockWait, "_caser_patched", False):
        return
    _orig_assign = TileClockWait.assign_waits

    def _assign_waits_split(self, start_bb):
        r = _orig_assign(self, start_bb)
        nc = current_nc[0]
        for _bb, insts in self.ordered_instructions_by_block.items():
            new = []
            for inst in insts:
                si = getattr(inst, "sync_info", None)
                if si is not None and si.on_wait and len(si.on_wait) > 1:
                    waits = list(si.on_wait)
                    for w in waits[:-1]:
                        new.append(
                            mybir.InstNoOp(
                                name=nc.get_next_instruction_name(),
                                engine=inst.engine,
                                ins=[],
                                outs=[],
                                sync_info=mybir.SyncInfo(on_wait=[w], on_update=[]),
                            )
                        )
                    si.on_wait = [waits[-1]]
                new.append(inst)
            insts[:] = new
        return r

    TileClockWait.assign_waits = _assign_waits_split
    TileClockWait._caser_patched = True

    def _patched_drain(self, tick_clock, wait_clock):
        nc = self.nc
        probe = nc.sync.nop()
        wait_clock.add_sem_waits(
            probe.ins, ScopedClock({None: tick_clock.global_clock})
        )
        si = probe.ins.sync_info
        waits = list(si.on_wait) if si is not None and si.on_wait else []
        if len(waits) > 1:
            si.on_wait = [waits[0]]
            for w in waits[1:]:
                n = nc.sync.nop()
                if n.ins.sync_info is None:
                    n.ins.sync_info = mybir.SyncInfo(on_wait=[w], on_update=[])
                else:
                    n.ins.sync_info.on_wait = [w]
        nc.sync.drain()
        nc.all_engine_barrier()
        assert self.sems is not None
        popped = nc._tile_sem_poison_stack.pop()
        assert popped is self._sem_poison
        nc.clear_and_free_semaphores(list(self.sems.allocated().values()))
        nc.all_engine_barrier()

    TileContext._drain_and_barrier = _patched_drain


def _build_program():
    import concourse.bass as bass
    import concourse.mybir as mybir
    from concourse.masks import make_identity
    from concourse.tile import ScopedClock, TileContext
    from concourse.tile_clock_wait import TileClockWait

    dt = mybir.dt
    f32, f16, i32 = dt.float32, dt.float16, dt.int32
    bf16 = f16  # fp16 everywhere (see _prep_host)
    AF = mybir.ActivationFunctionType
    AX = mybir.AxisListType
    OP = mybir.AluOpType

    _install_walrus_workarounds(mybir, TileContext, TileClockWait, ScopedClock)

    nc = bass.Bass(num_devices=NCORES, num_swdge_queues=SWDGE_QUEUES)
    _CURRENT_NC[0] = nc

    seq_i = nc.dram_tensor("seq_i", [BL, L], i32, kind="ExternalInput")
    uid_i = nc.dram_tensor("uid_i", [BL, 1], i32, kind="ExternalInput")
    item_emb = nc.dram_tensor("item_emb", [N_ITEMS, D], f32, kind="ExternalInput")
    user_emb = nc.dram_tensor("user_emb", [N_USERS, D], f32, kind="ExternalInput")
    wstack = nc.dram_tensor("wstack", [WSTACK_N, D, 128], bf16, kind="ExternalInput")
    mlhs = nc.dram_tensor("mlhs", [NG, D, 128], bf16, kind="ExternalInput")
    mrhs = nc.dram_tensor("mrhs", [NG, D, CONV_PSUM], bf16, kind="ExternalInput")
    gmat = nc.dram_tensor("gmat", [L, D, D], bf16, kind="ExternalInput")
    f1h = nc.dram_tensor("f1h", [NG, 128, D], bf16, kind="ExternalInput")
    f2z = nc.dram_tensor("f2z", [D, D], bf16, kind="ExternalInput")
    f2u = nc.dram_tensor("f2u", [D, D], bf16, kind="ExternalInput")
    biases = nc.dram_tensor("biases", [128, 9], f32, kind="ExternalInput")
    embt16 = nc.dram_tensor("embt16", [D, IPC], f16, kind="ExternalInput")
    out = nc.dram_tensor("out", [B, IPC], f32, kind="ExternalOutput")
    cc_in = [nc.dram_tensor(f"cc_in{s_}", [D, SB], f16) for s_ in range(NSLICE)]
    cc_out = [
        nc.dram_tensor(f"cc_out{s_}", [NCORES, D, SB], f16, addr_space="Shared")
        for s_ in range(NSLICE)
    ]
    if DEBUG_DUMPS:
        dbg_xt = nc.dram_tensor("dbg_xt", [D, BL * L], bf16, kind="ExternalOutput")
        dbg_pools = nc.dram_tensor(
            "dbg_pools", [NG, 128, BL], f16, kind="ExternalOutput"
        )
        dbg_zt = nc.dram_tensor("dbg_zt", [D, SB], bf16, kind="ExternalOutput")
        dbg_ut = nc.dram_tensor("dbg_ut", [D, BL], bf16, kind="ExternalOutput")
        dbg_so = nc.dram_tensor("dbg_so", [D, SB], f16, kind="ExternalOutput")

    with TileContext(nc) as tc:
        with (
            tc.tile_pool(name="const", bufs=1) as constp,
            tc.tile_pool(name="pools", bufs=2) as poolsp,
            tc.tile_pool(name="fcsb", bufs=2) as fcsbp,
            tc.tile_pool(name="ostage", bufs=3) as ostagep,
            tc.tile_pool(name="tr_ps", bufs=1, space="PSUM") as trpsp,
            tc.tile_pool(name="conv_ps", bufs=4, space="PSUM") as convpsp,
            tc.tile_pool(name="fc_ps", bufs=1, space="PSUM") as fcpsp,
            tc.tile_pool(name="log_ps", bufs=2, space="PSUM") as logpsp,
        ):
            ident = constp.tile([128, 128], f32)
            make_identity(nc, ident[:])
            bias_sb = constp.tile([128, 9], f32)
            nc.sync.dma_start(bias_sb[:], biases[:])

            # ---------------- Phase A: gathers + transpose to XT ----------
            # (the DMA device serializes transfers in issue order, so the
            # latency-critical gathers are emitted before the weight loads)
            idx_sb = constp.tile([BL, L], i32)
            nc.sync.dma_start(idx_sb[:], seq_i[:])
            uidx_sb = constp.tile([BL, 1], i32)
            nc.sync.dma_start(uidx_sb[:], uid_i[:])
            ug = constp.tile([BL, D], f32)
            nc.gpsimd.indirect_dma_start(
                out=ug[:],
                out_offset=None,
                in_=user_emb[:],
                in_offset=bass.IndirectOffsetOnAxis(ap=uidx_sb[:], axis=0),
            )

            xt = constp.tile([D, BL * L], bf16)
            xt3 = xt[:].rearrange("p (b t) -> p b t", t=L)  # [d, b, t]
            # per-l gather tiles so transpose l only waits on gather l
            # (HW indirect DMA gathers one row per partition per call)
            with tc.tile_pool(name="xgp", bufs=4) as xgp:
                for l in range(L):
                    xgl = xgp.tile([BL, D], f32, tag="xg", name=f"xg{l}")
                    nc.gpsimd.indirect_dma_start(
                        out=xgl[:],
                        out_offset=None,
                        in_=item_emb[:],
                        in_offset=bass.IndirectOffsetOnAxis(
                            ap=idx_sb[:, l : l + 1], axis=0
                        ),
                    )
                    tp = trpsp.tile([128, 128], f32)
                    nc.tensor.transpose(tp[:], xgl[:], ident[:])
                    nc.vector.tensor_copy(out=xt3[:, :, l], in_=tp[:])
            ut = constp.tile([D, BL], bf16)
            tpu = trpsp.tile([128, 128], f32, tag="tp")
            nc.tensor.transpose(tpu[:], ug[:], ident[:])
            nc.vector.tensor_copy(out=ut[:], in_=tpu[:])

            # ---- resident weights, loaded per group in the order the conv
            # pipeline consumes them (sliced groups first, hoisted later) ----
            ml_all = constp.tile([D, NG, 128], bf16)
            nc.sync.dma_start(ml_all[:], mlhs[:].rearrange("g d m -> d g m"))
            mr_all = constp.tile([D, NG, CONV_PSUM], bf16)
            nc.sync.dma_start(mr_all[:], mrhs[:].rearrange("g d c -> d g c"))
            wt_all = constp.tile([D, WSTACK_N, 128], bf16)
            for g_ in list(HOIST_G) + [g for g in range(NG) if g not in HOIST_G]:
                j0_, jn_ = WSTACK_STARTS[g_], GEO[g_][1]
                nc.sync.dma_start(
                    wt_all[:, j0_ : j0_ + jn_, :],
                    wstack[j0_ : j0_ + jn_].rearrange("j d m -> d j m"),
                )
            gm = constp.tile([D, L, D], bf16)
            nc.sync.dma_start(gm[:], gmat[:].rearrange("l d o -> d l o"))
            fh = constp.tile([128, NG, D], bf16)
            nc.sync.dma_start(fh[:], f1h[:].rearrange("g r o -> r g o"))
            wz = constp.tile([D, D], bf16)
            nc.sync.dma_start(wz[:], f2z[:])
            wu = constp.tile([D, D], bf16)
            nc.sync.dma_start(wu[:], f2u[:])
            esb = constp.tile([D, IPC], f16)
            nc.sync.dma_start(esb[:], embt16[:])

            # ---- zv: vertical-conv contribution to fc1, all 128 rows ----
            psv = fcpsp.tile([128, BL], f32, tag="fc", name="psv")
            for l in range(L):
                nc.tensor.matmul(
                    psv[:],
                    lhsT=gm[:, l, :],
                    rhs=xt3[:, :, l],
                    start=(l == 0),
                    stop=(l == L - 1),
                    skip_group_check=True,
                )
            zvb = constp.tile([D, BL], f32)
            nc.scalar.activation(zvb[:], psv[:], AF.Identity, bias=bias_sb[:, 7:8])

            def conv_group(g, gb0, gnb, praw_name):
                """Conv group g over local batch rows [gb0, gb0+gnb);
                returns the raw (pre-relu/bias) pooled tile [128, gnb]."""
                T, jmax = GEO[g]
                j0 = WSTACK_STARTS[g]
                praw = poolsp.tile(
                    [128, gnb], f32, tag=praw_name, name=f"{praw_name}_t"
                )
                cbmax = min(gnb, CONV_PSUM // T)
                nch = -(-gnb // cbmax)
                b0 = gb0
                ci = 0
                while b0 < gb0 + gnb:
                    cbe = (gb0 + gnb - b0 + (nch - ci) - 1) // (nch - ci)
                    ps = convpsp.tile([128, CONV_PSUM], f32)
                    ps3 = ps[:, : cbe * T].rearrange("p (b t) -> p b t", t=T)
                    for j in range(jmax):
                        Tj = L - max(j, 8 * g)  # == T for j <= 8g
                        nc.tensor.matmul(
                            ps3[:, :, :Tj],
                            lhsT=wt_all[:, j0 + j, :],
                            rhs=xt3[:, b0 : b0 + cbe, j : j + Tj],
                            start=(j == 0),
                            stop=False,
                            skip_group_check=True,
                        )
                    nc.tensor.matmul(
                        ps[:, : cbe * T],
                        lhsT=ml_all[:, g, :],
                        rhs=mr_all[:, g, : cbe * T],
                        start=False,
                        stop=True,
                        skip_group_check=True,
                    )
                    nc.vector.tensor_reduce(
                        out=praw[:, b0 - gb0 : b0 - gb0 + cbe],
                        in_=ps3,
                        axis=AX.X,
                        op=OP.max,
                    )
                    b0 += cbe
                    ci += 1
                    pump_logits(1)
                return praw

            hoisted = {}

            def emit_hoisted():
                for g in HOIST_G:
                    praw = conv_group(g, 0, BL, f"hpraw{g}")
                    prelu = poolsp.tile(
                        [128, BL], bf16, tag=f"hprelu{g}", name=f"hprelu{g}_t"
                    )
                    nc.scalar.activation(
                        prelu[:], praw[:], AF.Relu, bias=bias_sb[:, g : g + 1]
                    )
                    hoisted[g] = prelu

            # ---- sliced pipeline ----------------------------------------
            # logits work items are pumped one at a time between conv chunks
            # so a logits matmul never stalls the in-order PE stream waiting
            # for its PSUM-drain copy
            logits_queue = []
            dbg = {}
            copy_flip = [0]

            def emit_logits_item(sl, soall, m2, oc0):
                npairs = (NCORES * SB) // 128
                nrank = 128 // SB
                lhsT = soall[:, m2 * 128 : (m2 + 1) * 128]
                ow = min(OCHUNK, IPC - oc0)
                osb = ostagep.tile([128, OCHUNK], f32, tag="ost", name=f"osb{sl}_{m2}_{oc0}")
                for s0 in range(oc0, oc0 + ow, SUB):
                    sw = min(SUB, oc0 + ow - s0)
                    pl = logpsp.tile([128, SUB], f32)
                    nc.tensor.matmul(
                        pl[:, :sw],
                        lhsT=lhsT,
                        rhs=esb[:, s0 : s0 + sw],
                        start=True,
                        stop=True,
                    )
                    if copy_flip[0] % 2 == 0:
                        nc.scalar.copy(
                            out=osb[:, s0 - oc0 : s0 - oc0 + sw], in_=pl[:, :sw]
                        )
                    else:
                        nc.vector.tensor_copy(
                            out=osb[:, s0 - oc0 : s0 - oc0 + sw], in_=pl[:, :sw]
                        )
                    copy_flip[0] += 1
                dest = out[:].rearrange(
                    "(m2 r s f) i -> m2 r s f i",
                    m2=npairs, r=nrank, s=NSLICE,
                )[m2, :, sl, :, oc0 : oc0 + ow]
                nc.sync.dma_start(dest, osb[:, :ow])

            def pump_logits(n=1):
                for _ in range(n):
                    if not logits_queue:
                        return
                    args = logits_queue.pop(0)
                    emit_logits_item(*args)

            emit_hoisted()

            for sl in range(NSLICE):
                sb0 = sl * SB
                # ---------- conv (non-hoisted groups) for this slice -----
                pools_sb = {}
                for g in range(NG):
                    if g in HOIST_G:
                        continue
                    praw = conv_group(g, sb0, SB, f"praw{g}")
                    prelu = poolsp.tile(
                        [128, SB], bf16, tag=f"prelu{g}", name=f"prelu{sl}_{g}"
                    )
                    nc.scalar.activation(
                        prelu[:], praw[:], AF.Relu, bias=bias_sb[:, g : g + 1]
                    )
                    pools_sb[g] = prelu

                # ---------- fc1 + fc2 for this slice ---------------------
                psz = fcpsp.tile([128, SB], f32, tag="fc", name=f"psz{sl}")
                for gi, g in enumerate(range(NG)):
                    rhs = (
                        hoisted[g][:, sb0 : sb0 + SB]
                        if g in HOIST_G
                        else pools_sb[g][:]
                    )
                    nc.tensor.matmul(
                        psz[:],
                        lhsT=fh[:, g, :],
                        rhs=rhs,
                        start=(gi == 0),
                        stop=(gi == NG - 1),
                        skip_group_check=True,
                    )
                ztmp = fcsbp.tile([D, SB], f32, tag="ztmp", name=f"ztmp{sl}")
                nc.vector.tensor_add(
                    out=ztmp[:], in0=psz[:], in1=zvb[:, sb0 : sb0 + SB]
                )
                zt = fcsbp.tile([D, SB], bf16, tag="zt", name=f"zt{sl}")
                nc.scalar.activation(zt[:], ztmp[:], AF.Relu)

                pss = fcpsp.tile([128, SB], f32, tag="fc", name=f"pss{sl}")
                nc.tensor.matmul(
                    pss[:], lhsT=wz[:], rhs=zt[:], start=True, stop=False,
                    skip_group_check=True,
                )
                nc.tensor.matmul(
                    pss[:], lhsT=wu[:], rhs=ut[:, sb0 : sb0 + SB],
                    start=False, stop=True, skip_group_check=True,
                )
                so16 = fcsbp.tile([D, SB], f16, tag="so16", name=f"so16_{sl}")
                nc.scalar.activation(
                    so16[:], pss[:], AF.Relu, bias=bias_sb[:, 8:9]
                )
                if sl == 0:
                    dbg["zt"] = zt
                    dbg["so"] = so16

                # ---------- AllGather seq_out across the 8 cores ----------
                # keep these off the SP queue: a wait here would block the
                # logits out-DMA configs emitted later on SP
                nc.gpsimd.dma_start(cc_in[sl][:], so16[:])
                nc.gpsimd.collective_compute(
                    "AllGather",
                    mybir.AluOpType.bypass,
                    replica_groups=[list(range(NCORES))],
                    ins=[cc_in[sl][:]],
                    outs=[cc_out[sl][:]],
                )
                soall = fcsbp.tile(
                    [D, NCORES * SB], f16, tag="soall", name=f"soall{sl}"
                )
                nc.gpsimd.dma_start(
                    soall[:].rearrange("p (c f) -> p c f", f=SB),
                    cc_out[sl][:].rearrange("c p f -> p c f"),
                )

                # queue this slice's logits; items are pumped between the
                # next slices' conv chunks
                if not SKIP_LOGITS:
                    npairs = (NCORES * SB) // 128
                    for m2 in range(npairs):
                        for oc0 in range(0, IPC, OCHUNK):
                            logits_queue.append((sl, soall, m2, oc0))
            pump_logits(len(logits_queue))

            if DEBUG_DUMPS:
                nc.sync.dma_start(dbg_xt[:], xt[:])
                nc.sync.dma_start(dbg_zt[:], dbg["zt"][:])
                nc.sync.dma_start(dbg_ut[:], ut[:])
                nc.sync.dma_start(dbg_so[:], dbg["so"][:])

    return nc


_PROGRAM = None


def _get_program():
    global _PROGRAM
    if _PROGRAM is None:
        _PROGRAM = _build_program()
    return _PROGRAM


def kernel(**inputs):
    global LAST_EXEC_NS
    from concourse.bass_utils import run_bass_kernel_spmd

    nc = _get_program()
    in_maps = _prep_host(inputs)
    trace = bool(int(os.environ.get("CASER_TRACE", "0")))
    res = run_bass_kernel_spmd(
        nc, in_maps, core_ids=list(range(NCORES)), trace=trace
    )
    LAST_EXEC_NS = res.exec_time_ns
    if LAST_EXEC_NS is None:
        # NTFF profiling is unavailable under this axon setup; fall back to
        # the per-core cost-model timeline (the same model Tile schedules
        # against) for the reported kernel time.
        try:
            from concourse.timeline_sim import TimelineSim

            LAST_EXEC_NS = int(TimelineSim(nc, trace=False).simulate())
        except Exception:
            LAST_EXEC_NS = None
    return np.concatenate([res.results[c]["out"] for c in range(NCORES)], axis=1)
